# revision 21
# baseline (speedup 1.0000x reference)
"""Trainium2 Bass kernel for nn_Auto_Attn (self-attention + context flow + convs).

Sharding: 8 cores = (batch b in 0..3) x (image half s in 0..1). Each core
computes its half's 32 output rows plus 2 halo rows of the attention output
(conv1 3x3 -> conv2 3x3 needs a 2-row inp halo).

Per-core algorithm (validated against a float64 reference in numpy):
  q = wq @ x (1x1 conv);  energy[m,n] = q_m . q_n  (symmetric since key==query)
  Softmax rows are computed TRANSPOSED (eT[n,m], n on partitions) so the
  attention einsums need no transposes of the big attn matrix:
    - shift-invariance: softmax(e[m,:]) == softmax(e[m,:] - d[m]) for
      d[m] = e[m,m]; Cauchy-Schwarz bounds e[m,n]-d[m] <= (e[n,n]-e[m,m])/2
      so exp never overflows fp32. The shift is folded into the energy matmul
      via an augmented contraction row: q_aug_n = [q; 1], q_aug_m = [q; -d].
    - Z[m] arrives free as a 257th ones-column in the value matrix.

Pipelining: one fully-streamed loop over 288 (m-chunk, nt-pair) steps — m in
128-wide chunks (= 2 image rows), n in pairs of 128-tiles.  An energy pair
lands in one PSUM bank (second matmul start=False into the zeroed half), one
256-wide Exp moves it to SBUF, and the 4 value matmuls of the pair hide the
Exp.  Energy pairs are issued 2 steps ahead; the value accumulators are
double-buffered so consecutive m-chunks never wait on the epilogue PSUM
reads (any PE idle gap also drops the clock to 1.2GHz for 3us, so seams are
doubly expensive).  PSUM: 2 energy + 4 value + 2 transpose banks = 8.
Epilogue scalings (1/Z, gamma/alpha/mask; per-partition scalars in [m, c]
layout) and the PE-transposes to [c, m] are woven between the next chunk's
value matmuls; the x / (1-M)*pre additive terms are added post-transpose
from host-prepared windowed tiles (xm / pm2).

DMA: all large tensors are host-packed into their exact SBUF layout
[128 x cols]; a DMA trigger costs ~600ns of queue time, so transfers are
few and big.  Two hardware queues run in parallel: SP streams the q-path
inputs (xm, xb chunked so energy can start after the lower half), the
Activation queue streams the value matrix then the conv weights.

dtypes: bf16 everywhere on the PE (q path, energy, value, transposes,
convs) with f32 PSUM accumulation; fp32 only for the softmax scalings.
"""
import numpy as np
from contextlib import ExitStack

import ml_dtypes
import concourse.bass as bass
import concourse.tile as tile
from concourse import bacc, mybir
from concourse.bass_utils import run_bass_kernel_spmd
from concourse.masks import make_identity

F32 = mybir.dt.float32
F32R = mybir.dt.float32r
BF16 = mybir.dt.bfloat16

B, C, H, W = 4, 256, 64, 64
N = H * W                # 4096
CQ = 64
HALF = 32
WIN = 36                 # window rows (32 + 2 halo each side)
BUFR, BUFC = 38, 66      # padded conv buffer: +1 guard row / +1 pad col each side
MW = WIN * 64            # 2304 m positions per core
NT = N // 128            # 32 n-tiles
NPB = NT // 2            # nt-pairs per m-chunk
MBLK = 128               # attention m-chunk (2 image rows)
NBLK = MW // MBLK        # 18
VPT = 513                # [v(256) | ones(1) | pre(256)]
POS1_LO, POS1_HI = 2 * BUFC, 36 * BUFC     # conv1 output span (2244)
POS2_LO, POS2_HI = 3 * BUFC, 35 * BUFC     # conv2/out span (2112)
C1_CHUNKS = [512, 512, 512, 452, 256]      # sum 2244
C2_CHUNKS = [512, 512, 512, 320, 256]      # sum 2112

_PROGRAM_CACHE = {}


def tap_off(t):
    return (t // 3 - 1) * BUFC + (t % 3 - 1)


# --------------------------------------------------------------------------
# host-side prep
# --------------------------------------------------------------------------

def host_prep(inputs, core):
    b, s = core // 2, core % 2
    f32 = np.float32
    bf16 = ml_dtypes.bfloat16
    x = np.asarray(inputs['x'][b], f32).reshape(C, N)
    pre = np.asarray(inputs['pre'][b], f32).reshape(C, N)
    M = np.asarray(inputs['mask'][b, 0], f32).reshape(N)
    gamma = float(np.asarray(inputs['gamma']).reshape(-1)[0])
    alpha = float(np.asarray(inputs['alpha']).reshape(-1)[0])

    r_lo = s * HALF - 2
    rows = np.arange(r_lo, r_lo + WIN)
    valid_rows = (rows >= 0) & (rows < H)
    valid = np.repeat(valid_rows, 64).astype(f32)

    def win_slice(t):
        o = np.zeros((t.shape[0], MW), f32)
        vr = np.where(valid_rows)[0]
        o[:, vr[0] * 64:vr[-1] * 64 + 64] = t[:, rows[vr[0]] * 64: rows[vr[-1]] * 64 + 64]
        return o

    xm = win_slice(x)
    pm = win_slice(pre)
    Mw = win_slice(M[None, :])[0]

    vpT = np.zeros((N, VPT), f32)
    vpT[:, 0:C] = x.T
    vpT[:, C] = 1.0
    vpT[:, C + 1:VPT] = pre.T

    def sb(a, inner):  # [K*128, inner] -> SBUF layout [128, K*inner]
        k = a.shape[0] // 128
        return np.ascontiguousarray(
            a.reshape(k, 128, inner).transpose(1, 0, 2).reshape(128, k * inner))

    def pervec(v):  # [MW] -> [128, 18] (per-partition layout per m-chunk)
        return np.ascontiguousarray(v.reshape(NBLK, 128).T)

    wq = np.asarray(inputs['wq'], f32).reshape(CQ, C)
    w1 = np.asarray(inputs['w1'], f32)
    w2 = np.asarray(inputs['w2'], f32)
    ws = np.asarray(inputs['ws'], f32).reshape(C, 2 * C)
    bq = np.asarray(inputs['bq'], f32).reshape(CQ, 1)
    b1 = np.asarray(inputs['b1'], f32)
    b2 = np.asarray(inputs['b2'], f32)
    bs = np.asarray(inputs['bs'], f32)

    # conv weights in SBUF layout [128, (tap, in_chunk)*C]
    w1s = np.transpose(w1, (2, 3, 1, 0)).reshape(9, 4, 128, C)
    w1s = np.ascontiguousarray(w1s.transpose(2, 0, 1, 3).reshape(128, 36 * C))
    w2s = np.transpose(w2, (2, 3, 1, 0)).reshape(9, 2, 128, C)
    w2s = np.ascontiguousarray(w2s.transpose(2, 0, 1, 3).reshape(128, 18 * C))
    wss = sb(ws.T.copy(), C)

    br_rows = r_lo + np.arange(BUFR) - 1
    rv = ((br_rows >= 0) & (br_rows < H)).astype(f32)
    cv = np.zeros(BUFC, f32)
    cv[1:65] = 1.0
    hrmask = np.broadcast_to((rv[:, None] * cv[None, :]).reshape(1, -1),
                             (128, BUFR * BUFC))

    return {
        'xb': sb(x, N).astype(bf16),
        'xm': sb(xm, MW).astype(bf16),
        'pm2': sb((1.0 - Mw) * valid * pm, MW).astype(bf16),
        'vpT': sb(vpT.reshape(NT * 128, VPT), VPT).astype(bf16),
        'avec': pervec(gamma * valid),
        'b1vec': pervec(alpha * Mw * valid),
        'wq': sb(np.ascontiguousarray(wq.T), CQ).astype(bf16),
        'bq': bq,
        'w1': w1s.astype(bf16),
        'w2': w2s.astype(bf16),
        'ws': wss.astype(bf16),
        'b1c': np.ascontiguousarray(b1.reshape(2, 128).T),
        'bfin': np.ascontiguousarray((b2 + bs).reshape(2, 128).T),
        'hrmask': np.ascontiguousarray(hrmask).astype(bf16),
    }


# --------------------------------------------------------------------------
# device program
# --------------------------------------------------------------------------

def build_program():
    nc = bacc.Bacc("TRN2", target_bir_lowering=False, debug=False, num_devices=8)

    def din(name, shape, dt=F32):
        return nc.dram_tensor(name, shape, dt, kind="ExternalInput").ap()

    xb_d = din('xb', [128, 2 * N], BF16)
    xm_d = din('xm', [128, 2 * MW], BF16)
    pm2_d = din('pm2', [128, 2 * MW], BF16)
    vpT_d = din('vpT', [128, NT * VPT], BF16)
    avec_d = din('avec', [128, NBLK])
    b1vec_d = din('b1vec', [128, NBLK])
    wq_d = din('wq', [128, 2 * CQ], BF16)
    bq_d = din('bq', [CQ, 1])
    w1_d = din('w1', [128, 36 * C], BF16)
    w2_d = din('w2', [128, 18 * C], BF16)
    ws_d = din('ws', [128, 4 * C], BF16)
    b1c_d = din('b1c', [128, 2])
    bfin_d = din('bfin', [128, 2])
    hrmask_d = din('hrmask', [128, BUFR * BUFC], BF16)
    y_d = nc.dram_tensor('y', [2, 128, POS2_HI - POS2_LO], BF16,
                         kind="ExternalOutput").ap()

    with tile.TileContext(nc) as tc, ExitStack() as ctx:
        # ---------------- persistent tiles ----------------
        persist = ctx.enter_context(tc.tile_pool(name="persist", bufs=1))
        inp_raw = [persist.tile([128, BUFR * BUFC], BF16, tag=f"inp{i}", name=f"inp{i}")
                   for i in range(4)]
        h1 = [persist.tile([128, BUFR * BUFC], BF16, tag=f"h1{i}", name=f"h1{i}")
              for i in range(2)]
        w1_t = persist.tile([128, 36 * C], BF16, tag="w1")
        w2_t = persist.tile([128, 18 * C], BF16, tag="w2")
        ws_t = persist.tile([128, 4 * C], BF16, tag="ws")
        b1c_t = persist.tile([128, 2], F32, tag="b1c")
        bfin_t = persist.tile([128, 2], F32, tag="bfin")
        hrm_t = persist.tile([128, BUFR * BUFC], BF16, tag="hrm")
        out_sb = [persist.tile([128, POS2_HI - POS2_LO], BF16, tag=f"os{i}", name=f"os{i}")
                  for i in range(2)]
        # vpT / q_aug_n are split into per-group tiles: DMA->read dependencies
        # are tile-granular, so a monolithic tile would stall the first value
        # matmul until the LAST group's DMA lands.
        vpt = [persist.tile([128, 4 * VPT], BF16, tag=f"vpT{g}", name=f"vpT{g}")
               for g in range(8)]
        xm_t = persist.tile([128, 2 * MW], BF16, tag="xm")
        pm2_t = persist.tile([128, 2 * MW], BF16, tag="pm2")
        avec_t = persist.tile([128, NBLK], F32, tag="av")
        b1vec_t = persist.tile([128, NBLK], F32, tag="b1v")
        wq_t = persist.tile([128, 2 * CQ], BF16, tag="wq")
        bq_t = persist.tile([CQ, 1], F32, tag="bq")
        ident = persist.tile([128, 128], BF16, tag="id")
        qn = [persist.tile([CQ + 1, 512], BF16, tag=f"qn{k}", name=f"qn{k}")
              for k in range(8)]
        q_aug_m = persist.tile([CQ + 1, MW], BF16, tag="qm")
        expT = persist.tile([128, NT * MBLK], BF16, tag="expT")

        make_identity(nc, ident[:])
        # conv buffer border zeroing: cols 0 and 65 of every row (the only
        # positions the conv taps read that the attention epilogue doesn't
        # write); h1 additionally needs positions 131 / 2376 (pad-col taps).
        for t_ in inp_raw:
            a = t_[:].rearrange("p (r c) -> p r c", c=BUFC)
            nc.gpsimd.memset(a[:, :, 0:1], 0.0)
            nc.gpsimd.memset(a[:, :, 65:66], 0.0)
        for t_ in h1:
            nc.gpsimd.memset(t_[:, POS1_LO - 1:POS1_LO], 0.0)
            nc.gpsimd.memset(t_[:, POS1_HI:POS1_HI + 1], 0.0)

        # SP-queue DMAs: small constants first; the q-path inputs and the
        # value matrix follow below, interleaved so each lands just before
        # its first consumer.  The conv-phase constants are deferred into the
        # attention stream (ACT queue) so they don't steal early bandwidth.
        nc.sync.dma_start(bq_t[:], bq_d)
        nc.sync.dma_start(avec_t[:], avec_d)
        nc.sync.dma_start(b1vec_t[:], b1vec_d)
        nc.sync.dma_start(wq_t[:], wq_d)
        for k in range(8):
            nc.gpsimd.memset(qn[k][CQ:CQ + 1, :], 1.0)

        def dma_weights():
            nc.scalar.dma_start(w1_t[:], w1_d)
            nc.scalar.dma_start(w2_t[:], w2_d)
            nc.scalar.dma_start(ws_t[:], ws_d)
            nc.scalar.dma_start(hrm_t[:], hrmask_d)
            nc.scalar.dma_start(b1c_t[:], b1c_d)
            nc.scalar.dma_start(bfin_t[:], bfin_d)

        def dma_vpt_group(g):
            nc.sync.dma_start(vpt[g][:], vpT_d[:, g * 4 * VPT:(g + 1) * 4 * VPT])

        # ---------------- q phase ----------------
        with tc.tile_pool(name="qp", bufs=1) as qp, \
             tc.tile_pool(name="qps", bufs=2, space="PSUM") as qps:
            nc.sync.dma_start(xm_t[:], xm_d)
            mchunks = [512, 512, 512, 512, 256]
            qsq = qp.tile([CQ, MW], BF16, tag="qsq")
            mo = 0
            for cs in mchunks:
                qsum = qps.tile([CQ, 512], F32, tag="qpsum")
                for ck in range(2):
                    nc.tensor.matmul(qsum[:, 0:cs], wq_t[:, ck * CQ:(ck + 1) * CQ],
                                     xm_t[:, ck * MW + mo:ck * MW + mo + cs],
                                     start=(ck == 0), stop=(ck == 1))
                nc.scalar.activation(q_aug_m[0:CQ, mo:mo + cs], qsum[:, 0:cs],
                                     mybir.ActivationFunctionType.Identity,
                                     bias=bq_t[:])
                nc.vector.tensor_mul(qsq[:, mo:mo + cs], q_aug_m[0:CQ, mo:mo + cs],
                                     q_aug_m[0:CQ, mo:mo + cs])
                mo += cs

            # d[m] = sum_cq q_m^2 via ones-matmul on q_m^2
            ones_t = qp.tile([CQ, 1], BF16, tag="ones")
            nc.vector.memset(ones_t[:], 1.0)
            mo = 0
            for cs in mchunks:
                dps = qps.tile([1, 512], F32, tag="dpsum")
                nc.tensor.matmul(dps[:, 0:cs], ones_t[:],
                                 qsq[:, mo:mo + cs], start=True, stop=True)
                nc.scalar.activation(q_aug_m[CQ:CQ + 1, mo:mo + cs], dps[:, 0:cs],
                                     mybir.ActivationFunctionType.Identity,
                                     scale=-1.0)
                mo += cs

            # q over the full image; xb is chunked so the lower half (the
            # first 16 n-tiles) lands first and the energy stream can start;
            # the first value-matrix groups are interleaved between the xb
            # halves to match block 0's consumption order.
            # xb and the first value-matrix groups ride the ACT queue: the
            # sync engine serializes DMA triggers at ~600ns each, so splitting
            # the early triggers across both hardware queues shortens the
            # startup critical path.
            xb4 = {}
            for hi in range(2):
                if hi == 1:
                    nc.scalar.dma_start(vpt[0][:], vpT_d[:, 0:4 * VPT])
                    nc.scalar.dma_start(vpt[1][:], vpT_d[:, 4 * VPT:8 * VPT])
                for ck in range(2):
                    xb4[hi, ck] = qp.tile([128, 2048], BF16, tag=f"xb{hi}{ck}",
                                          name=f"xb{hi}{ck}")
                    nc.scalar.dma_start(
                        xb4[hi, ck][:],
                        xb_d[:, ck * N + hi * 2048:ck * N + (hi + 1) * 2048])
                for nb in range(4 * hi, 4 * hi + 4):
                    qsum = qps.tile([CQ, 512], F32, tag="qpsum")
                    for ck in range(2):
                        nc.tensor.matmul(
                            qsum[:], wq_t[:, ck * CQ:(ck + 1) * CQ],
                            xb4[hi, ck][:, (nb % 4) * 512:(nb % 4 + 1) * 512],
                            start=(ck == 0), stop=(ck == 1))
                    nc.scalar.activation(qn[nb][0:CQ, :], qsum[:],
                                         mybir.ActivationFunctionType.Identity,
                                         bias=bq_t[:])
            for g in range(2, 8):
                dma_vpt_group(g)
            nc.sync.dma_start(pm2_t[:], pm2_d)

        # ---------------- attention: one stream over (m-chunk, nt-pair) ----
        with tc.tile_pool(name="eps", bufs=2, space="PSUM") as eps, \
             tc.tile_pool(name="pop", bufs=2, space="PSUM") as pop, \
             tc.tile_pool(name="tps", bufs=2, space="PSUM") as tps, \
             tc.tile_pool(name="epp", bufs=2) as epp:

            def emit_energy_quad(blk, k):
                m0 = blk * MBLK
                ets = eps.tile([128, 4 * MBLK], F32, tag="ets")
                for i in range(4):
                    nc.tensor.matmul(ets[:, i * MBLK:(i + 1) * MBLK],
                                     qn[k][:, i * 128:(i + 1) * 128],
                                     q_aug_m[:, m0:m0 + MBLK],
                                     start=(i == 0), stop=(i == 3),
                                     skip_group_check=True)
                nc.scalar.activation(expT[:, 4 * k * MBLK:(4 * k + 4) * MBLK], ets[:],
                                     mybir.ActivationFunctionType.Exp)

            def emit_value(po, nt):
                lhs = expT[:, nt * MBLK:(nt + 1) * MBLK]
                g, o = nt // 4, (nt % 4) * VPT
                nc.tensor.matmul(po[0][:], lhs, vpt[g][:, o:o + 257],
                                 start=(nt == 0), stop=(nt == NT - 1))
                nc.tensor.matmul(po[1][:], lhs, vpt[g][:, o + 257:o + 513],
                                 start=(nt == 0), stop=(nt == NT - 1))

            def emit_epi_scalings(po, j):
                po_a, po_b = po
                rt = epp.tile([128, 1], F32, tag="rt")
                s1 = epp.tile([128, 1], F32, tag="s1")
                s2 = epp.tile([128, 1], F32, tag="s2")
                nc.vector.reciprocal(rt[:], po_a[:, 256:257])
                nc.vector.tensor_mul(s1[:], rt[:], avec_t[:, j:j + 1])
                nc.vector.tensor_mul(s2[:], rt[:], b1vec_t[:, j:j + 1])
                ov = epp.tile([128, C], BF16, tag="ov")
                cv = epp.tile([128, C], BF16, tag="cv")
                nc.vector.tensor_scalar_mul(ov[:], po_a[:, 0:C], s1[:])
                nc.vector.tensor_scalar_mul(cv[:], po_b[:, 0:C], s2[:])
                return (ov, cv, j)

            def emit_epi_half(pend, half):
                ov, cv, j = pend
                br0 = 1 + 2 * j
                for h_ in (2 * half, 2 * half + 1):
                    src = (ov if h_ < 2 else cv)
                    add_t = (xm_t if h_ < 2 else pm2_t)
                    ck = h_ % 2
                    pt = tps.tile([128, 128], BF16, tag="pt")
                    nc.tensor.transpose(pt[:], src[:, ck * 128:ck * 128 + 128],
                                        ident[:])
                    dst = inp_raw[h_][:].rearrange("p (r c) -> p r c", c=BUFC)
                    nc.vector.tensor_add(
                        dst[:, br0:br0 + 2, 1:65],
                        pt[:].rearrange("p (r c) -> p r c", c=64),
                        add_t[:, ck * MW + j * 128:ck * MW + (j + 1) * 128]
                        .rearrange("p (r c) -> p r c", c=64))

            quads = [(blk, k) for blk in range(NBLK) for k in range(NT // 4)]
            po_of = {}

            def get_po(blk):
                if blk not in po_of:
                    po_of[blk] = (pop.tile([128, 257], F32, tag="poa", name="poa"),
                                  pop.tile([128, 256], F32, tag="pob", name="pob"))
                return po_of[blk]

            emit_energy_quad(*quads[0])
            emit_energy_quad(*quads[1])
            pend = None
            for g, (blk, k) in enumerate(quads):
                po = get_po(blk)
                if k == 0 and blk > 0:
                    pend = emit_epi_scalings(get_po(blk - 1), blk - 1)
                for i in range(4):
                    emit_value(po, 4 * k + i)
                if g + 2 < len(quads):
                    emit_energy_quad(*quads[g + 2])
                if g == 12:
                    dma_weights()
                if pend is not None and k == 0:
                    emit_epi_half(pend, 0)
                elif pend is not None and k == 1:
                    emit_epi_half(pend, 1)
                    pend = None
            pend = emit_epi_scalings(get_po(NBLK - 1), NBLK - 1)
            emit_epi_half(pend, 0)
            emit_epi_half(pend, 1)

        # ---------------- conv phase ----------------
        with tc.tile_pool(name="lrp", bufs=2) as lrp, \
             tc.tile_pool(name="c1ps", bufs=2, space="PSUM") as c1ps, \
             tc.tile_pool(name="c2ps", bufs=2, space="PSUM") as c2ps:
            # conv1: h1 = lrelu(conv(lrelu(inp), w1) + b1) * mask
            base = POS1_LO
            for cs in C1_CHUNKS:
                lr = [lrp.tile([128, 646], BF16, tag=f"lr{i}", name=f"lr{i}")
                      for i in range(4)]
                for ick in range(4):
                    nc.scalar.activation(lr[ick][:, 0:cs + 134],
                                         inp_raw[ick][:, base - 67:base + cs + 67],
                                         mybir.ActivationFunctionType.Prelu,
                                         alpha=0.1)
                for oc in range(2):
                    ps1 = c1ps.tile([128, 512], F32, tag="ps1")
                    k = 0
                    for t in range(9):
                        off = 67 + tap_off(t)
                        for ick in range(4):
                            nc.tensor.matmul(
                                ps1[:, 0:cs],
                                w1_t[:, (t * 4 + ick) * C + oc * 128:(t * 4 + ick) * C + oc * 128 + 128],
                                lr[ick][:, off:off + cs],
                                start=(k == 0), stop=(k == 35))
                            k += 1
                    nc.scalar.activation(h1[oc][:, base:base + cs], ps1[:, 0:cs],
                                         mybir.ActivationFunctionType.Prelu,
                                         bias=b1c_t[:, oc:oc + 1], alpha=0.1)
                    nc.vector.tensor_mul(h1[oc][:, base:base + cs],
                                         h1[oc][:, base:base + cs],
                                         hrm_t[:, base:base + cs])
                base += cs

            # conv2 + shortcut
            base = POS2_LO
            for cs in C2_CHUNKS:
                for oc in range(2):
                    ps2 = c2ps.tile([128, 512], F32, tag="ps2")
                    k = 0
                    for ick in range(4):
                        nc.tensor.matmul(ps2[:, 0:cs],
                                         ws_t[:, ick * C + oc * 128:ick * C + oc * 128 + 128],
                                         inp_raw[ick][:, base:base + cs],
                                         start=(k == 0), stop=False)
                        k += 1
                    for t in range(9):
                        off = tap_off(t)
                        for ick in range(2):
                            k += 1
                            nc.tensor.matmul(
                                ps2[:, 0:cs],
                                w2_t[:, (t * 2 + ick) * C + oc * 128:(t * 2 + ick) * C + oc * 128 + 128],
                                h1[ick][:, base + off:base + off + cs],
                                start=False, stop=(k == 22))
                    nc.scalar.activation(out_sb[oc][:, base - POS2_LO:base - POS2_LO + cs],
                                         ps2[:, 0:cs],
                                         mybir.ActivationFunctionType.Identity,
                                         bias=bfin_t[:, oc:oc + 1])
                    nc.sync.dma_start(y_d[oc][:, base - POS2_LO:base - POS2_LO + cs],
                                      out_sb[oc][:, base - POS2_LO:base - POS2_LO + cs])
                base += cs

    nc.compile()
    return nc


# --------------------------------------------------------------------------
# entry point
# --------------------------------------------------------------------------

def _get_program():
    if 'nc' not in _PROGRAM_CACHE:
        _PROGRAM_CACHE['nc'] = build_program()
    return _PROGRAM_CACHE['nc']


def kernel(_trace=False, **inputs):
    nc = _get_program()
    in_maps = [host_prep(inputs, core) for core in range(8)]
    res = run_bass_kernel_spmd(nc, in_maps, core_ids=list(range(8)),
                               trace=_trace)
    y = np.zeros((B, C, H, W), np.float32)
    for core in range(8):
        b, s = core // 2, core % 2
        yh = np.asarray(res.results[core]['y'], dtype=np.float32)
        yh = yh.reshape(2, 128, HALF, BUFC)[:, :, :, 1:65]
        y[b, :, s * HALF:(s + 1) * HALF, :] = yh.reshape(C, HALF, 64)
    if _trace:
        return y, res
    return y
